# revision 4
# baseline (speedup 1.0000x reference)
"""Trainium2 Bass kernel for nn_DeChunkLayer (ragged_sequence).

Math (per batch row):
    p[c]     = clip(boundary_prob[take_idx[c]], EPS, 1-EPS)
    decay    = 1 - p, decay[0] = 0
    weighted = p * hidden, weighted[0] = hidden[0]
    smoothed[c] = decay[c] * smoothed[c-1] + weighted[c]      (EMA scan over C)
    chunk_id = clip(cumsum(boundary_mask) - 1, 0, C-1)
    out[l]   = smoothed[chunk_id[l]]

v3 vs v2:
  - The 8 output dma_gathers are split into PREPARE_ONLY descriptor
    generation (runs on GpSimd during phase 1, right after the p-gather)
    and per-call trigger_dma fires gated on the smoothed writeback.  This
    removes ~70us of serialized desc-gen from the phase-2 critical path
    (DMAGatherAnt desc-gen costs ~8.7ns/descriptor on HW).
  - The chunk-id computation is emitted before the scans so its DVE/PE
    work runs in the first ~12us (the gather preps need ckrep early).
  - p broadcast: [128, C] = ones[1,128].T @ p_row[1,128-block] outer
    products straight into PSUM (16 K=1 matmuls), replacing the
    DRAM-bounce broadcast.  decay = 1-p comes off PSUM on the Scalar
    engine; hidden is pre-cast to bf16 on the Scalar engine so the DVE
    multiply runs at 16-bit rate.
  - smoothed stays bf16 end-to-end as in v2 (fp32 scan state internally).
Tile does not emit RAW waits through DRAM scratch tensors, so the
triggers carry explicit sync deps on the smoothed writeback DMAs.
"""

import numpy as np

import concourse.bass as bass
import concourse.bacc as bacc
import concourse.mybir as mybir
import concourse.tile as tile
from concourse.bass_utils import run_bass_kernel_spmd

B, L, C, D = 8, 8192, 2048, 512
EPS = 1e-4
P = 128
NDG = D // P          # 4 partition groups of the transposed layout
NCB = C // P          # 16 c-blocks of 128
NPJ = C // P          # 16 p-gather calls
QW = 16               # wrap width of the dma_gather index layout
FW = L // QW          # 512 free positions in the wrapped layout
NGC = 8               # output dma_gather calls (num_idxs=1024 is a HW limit)
IPC = L // NGC        # 1024 indices per call
CH = C // 2           # broadcast half width

F32 = mybir.dt.float32
BF16 = mybir.dt.bfloat16
I16 = mybir.dt.int16
I32 = mybir.dt.int32
U8 = mybir.dt.uint8
COPY = mybir.ActivationFunctionType.Copy

_CACHED_NC = None


def build_nc() -> bacc.Bacc:
    nc = bacc.Bacc("TRN2", target_bir_lowering=False, debug=False)

    hidden_t = nc.dram_tensor("hidden_t", [D, C], F32, kind="ExternalInput")
    maskw_d = nc.dram_tensor("maskw", [QW, FW], U8, kind="ExternalInput")
    prob_d = nc.dram_tensor("prob", [L, 1], F32, kind="ExternalInput")
    tidx_d = nc.dram_tensor("tidx", [P, NPJ], I32, kind="ExternalInput")
    ident_d = nc.dram_tensor("ident", [P, P], F32, kind="ExternalInput")
    identbf_d = nc.dram_tensor("identbf", [P, P], BF16, kind="ExternalInput")
    uti_d = nc.dram_tensor("uti", [QW, QW], F32, kind="ExternalInput")
    out_d = nc.dram_tensor("out", [L, D], F32, kind="ExternalOutput")

    with tile.TileContext(nc) as tc:
        with (
            tc.tile_pool(name="persist", bufs=1) as pp,
            tc.tile_pool(name="hidden", bufs=2) as hp,
            tc.tile_pool(name="gather", bufs=NGC) as gp,
            tc.tile_pool(name="gout", bufs=2) as gop,
            tc.tile_pool(name="psum", bufs=2, space="PSUM") as psp,
        ):
            # psone (ck16 + p-row PSUM) closes before the 4-bank pb tile
            # is allocated (8 PSUM banks total).
            psone_cm = tc.tile_pool(name="psone", bufs=1, space="PSUM")
            psone = psone_cm.__enter__()
            # manual semaphores for the prepared output gathers (one DMA
            # sem per gather: the 16 per-queue completion incs of call k
            # don't distinguish calls on a shared counter). alloc does not
            # clear, so clear them up front on the (idle) sync engine.
            dma_sems = [nc.alloc_semaphore(f"gdma{k}") for k in range(NGC)]
            for s in dma_sems:
                nc.sync.sem_clear(s)

            # ---------------- input loads ----------------
            tidx_sb = pp.tile([P, NPJ], I32)
            nc.sync.dma_start(tidx_sb[:], tidx_d[:])
            maskw_sb = pp.tile([QW, FW], U8)
            nc.scalar.dma_start(maskw_sb[:], maskw_d[:])

            # constants come from the host (keeps GpSimd free for the
            # p gather + output-gather descriptor preps)
            ident = pp.tile([P, P], F32)
            nc.sync.dma_start(ident[:], ident_d[:])
            ident_bf = pp.tile([P, P], BF16)
            nc.sync.dma_start(ident_bf[:], identbf_d[:])
            uti = pp.tile([QW, QW], F32)
            nc.sync.dma_start(uti[:], uti_d[:])

            # ---------------- gpsimd: p gather ----------------
            p2 = pp.tile([P, NPJ], F32)   # (p, j) = p value for c = j*128 + p
            for j in range(NPJ):
                nc.gpsimd.indirect_dma_start(
                    out=p2[:, j:j + 1], out_offset=None, in_=prob_d[:],
                    in_offset=bass.IndirectOffsetOnAxis(
                        ap=tidx_sb[:, j:j + 1], axis=0))

            ones16 = pp.tile([1, QW], F32)
            nc.vector.memset(ones16[:], 1.0)
            ones161 = pp.tile([QW, 1], F32)
            nc.vector.memset(ones161[:], 1.0)
            ones1 = pp.tile([1, FW], F32)
            nc.vector.memset(ones1[:], 1.0)

            # ---------------- chunk ids (early: preps need ckrep) ----------
            # CK16[q, j] = chunk_id[16j + q]. Host passes maskw[q, j] =
            # mask[16j + q]. Within-column inclusive prefix over q via the
            # 16x16 inclusive triangular matmul; across-column exclusive
            # prefix of the column sums via a 1-partition scan, broadcast
            # into the same PSUM bank with a ones matmul.
            maskwf = pp.tile([QW, FW], F32)
            nc.vector.tensor_copy(maskwf[:], maskw_sb[:])
            ps16 = psone.tile([QW, FW], F32, space="PSUM", tag="ps16")
            nc.tensor.matmul(ps16[:], lhsT=uti[:], rhs=maskwf[:],
                             start=True, stop=False)
            cs_ps = psone.tile([1, FW], F32, space="PSUM", tag="cs")
            nc.tensor.matmul(cs_ps[:], lhsT=ones161[:], rhs=maskwf[:],
                             start=True, stop=True)
            colsb = pp.tile([1, FW], F32)
            nc.vector.tensor_copy(colsb[:], cs_ps[:])
            exc0 = pp.tile([1, FW], F32)
            nc.vector.tensor_tensor_scan(
                exc0[:], ones1[:], colsb[:],
                0.0, mybir.AluOpType.mult, mybir.AluOpType.add)
            nc.vector.tensor_tensor(exc0[:], exc0[:], colsb[:],
                                    mybir.AluOpType.subtract)
            nc.tensor.matmul(ps16[:], lhsT=ones16[:], rhs=exc0[:],
                             start=False, stop=True)
            ck = pp.tile([QW, FW], F32)
            nc.vector.tensor_scalar(ck[:], ps16[:], -1.0, None,
                                    mybir.AluOpType.add)
            nc.vector.tensor_scalar(ck[:], ck[:], 0.0, float(C - 1),
                                    mybir.AluOpType.max, mybir.AluOpType.min)
            ck16 = pp.tile([QW, FW], I16)
            nc.vector.tensor_copy(ck16[:], ck[:])
            # replicate to all 8 GPSIMD core groups (cross-partition copies)
            ckrep = pp.tile([P, FW], I16)
            for cgrp in range(P // QW):
                nc.scalar.dma_start(ckrep[cgrp * QW:(cgrp + 1) * QW, :], ck16[:])

            # prepared output gathers: desc-gen on GpSimd NOW (only needs
            # ckrep); the DMAs fire at the trigger below. sm_scratch is a
            # raw (untracked) DRAM tensor so the preps do not inherit the
            # smoothed-writeback RAW dep -- that dep is carried manually
            # by the trigger.
            sm_dram = nc.dram_tensor("sm_scratch", [C, D], BF16, kind="Internal")
            gs = []

            def emit_prep(k):
                g = gp.tile([P, IPC // P, D], BF16, tag="g")
                nc.gpsimd.dma_gather(
                    out_ap=g[:], in_ap=sm_dram[:],
                    idxs_ap=ckrep[:, k * (FW // NGC):(k + 1) * (FW // NGC)],
                    num_idxs=IPC, num_idxs_reg=IPC, elem_size=D,
                    prepare_only=True, sem=dma_sems[k])
                gs.append(g)

            NEARLY = 5
            for k in range(NEARLY):
                emit_prep(k)

            # ---------------- p row -> PSUM broadcast ----------------
            nc.vector.tensor_scalar(p2[:], p2[:], EPS, 1.0 - EPS,
                                    mybir.AluOpType.max, mybir.AluOpType.min)
            nc.vector.memset(p2[0:1, 0:1], 1.0)   # weighted[0]=hidden[0], decay[0]=0
            pT_ps = psone.tile([NPJ, P], F32, space="PSUM", tag="prow")
            nc.tensor.transpose(pT_ps[:], p2[:], ident[:])
            pT = pp.tile([NPJ, P], F32)
            nc.scalar.copy(pT[:], pT_ps[:])
            prow = pp.tile([1, C], F32)
            p_dram = nc.dram_tensor("p_row_scratch", [1, C], F32, kind="Internal")
            w_pr = nc.scalar.dma_start(
                p_dram[:].rearrange("o (j q) -> (o j) q", j=NPJ), pT[:])
            r_pr = nc.scalar.dma_start(prow[:], p_dram[:])
            bass._add_dep_helper(r_pr.ins, w_pr.ins, sync=True,
                                 reason="p row bounce raw")
            ones128 = pp.tile([1, P], F32)
            nc.vector.memset(ones128[:], 1.0)
            psone_cm.__exit__(None, None, None)
            pbp_cm = tc.tile_pool(name="pbh", bufs=1, space="PSUM")
            pbp = pbp_cm.__enter__()
            pb_ps = pbp.tile([P, C], F32, space="PSUM", tag="pb")
            for n in range(C // 512):
                nc.tensor.matmul(pb_ps[:, n * 512:(n + 1) * 512],
                                 lhsT=ones128[:],
                                 rhs=prow[:, n * 512:(n + 1) * 512],
                                 start=True, stop=True)
            db = pp.tile([P, C], BF16)
            nc.scalar.activation(db[:], pb_ps[:], COPY, bias=1.0, scale=-1.0)

            # ---------------- EMA scan in transposed layout ----------------
            sm_sb = pp.tile([P, NCB * D], BF16)  # [c-in-block, (c-block, d)]
            for dg in range(NDG):
                ht = hp.tile([P, C], F32, tag="ht")
                nc.sync.dma_start(ht[:], hidden_t[dg * P:(dg + 1) * P, :])
                wt = pp.tile([P, C], BF16, tag="wt")
                nc.vector.tensor_tensor(wt[:], ht[:], pb_ps[:],
                                        mybir.AluOpType.mult)
                st = pp.tile([P, C], BF16, tag=f"st{dg}")
                nc.vector.tensor_tensor_scan(
                    st[:], db[:], wt[:], 0.0,
                    mybir.AluOpType.mult, mybir.AluOpType.add)
                # transpose this dg's 16 c-blocks while the next scan runs;
                # copies go to Scalar (DVE keeps scanning), except a couple
                # on DVE after the last scan.
                for ci in range(NCB):
                    ps = psp.tile([P, P], BF16, space="PSUM", tag="tps")
                    nc.tensor.transpose(ps[:], st[:, ci * P:(ci + 1) * P],
                                        ident_bf[:])
                    dst = sm_sb[:, ci * D + dg * P: ci * D + (dg + 1) * P]
                    if dg == NDG - 1 and ci % 2 == 0:
                        nc.vector.tensor_copy(dst, ps[:])
                    else:
                        nc.scalar.copy(dst, ps[:])

            # split smoothed writeback so the first gather can start after
            # the first half (tokens of call k only reference c < 1024(k+1))
            sm_v = sm_dram[:].rearrange("(ci p) d -> p ci d", p=P)
            sb_v = sm_sb[:].rearrange("p (ci d) -> p ci d", d=D)
            HB = NCB // 2
            w_sm_a = nc.sync.dma_start(sm_v[:, 0:HB, :], sb_v[:, 0:HB, :])
            w_sm_b = nc.sync.dma_start(sm_v[:, HB:NCB, :], sb_v[:, HB:NCB, :])

            # ---------------- output expansion ----------------
            # Descriptor generation happens on GpSimd right after the
            # p-gather (PREPARE_ONLY); the per-call triggers fire once the
            # smoothed writeback lands.
            # fire the first NEARLY gathers once the writeback lands (their
            # preps are already done by then); the remaining preps interleave
            # with their own triggers so phase-2 DMA overlaps desc-gen.
            trig_of = []

            def emit_trigger(count, nprep):
                tr = nc.gpsimd.trigger_dma(count=count)
                bass._add_dep_helper(tr.ins, w_sm_a.ins, sync=True,
                                     reason="smoothed gather raw a")
                bass._add_dep_helper(tr.ins, w_sm_b.ins, sync=True,
                                     reason="smoothed gather raw b")
                return tr

            tr0 = emit_trigger(NEARLY, NEARLY)
            trig_of.extend([tr0] * NEARLY)
            for k in range(NEARLY, NGC):
                emit_prep(k)
                trig_of.append(emit_trigger(1, k + 1))

            for k in range(NGC):
                go = gop.tile([P, IPC // P, D], F32, tag="go")
                if k % 2 == 0:
                    cp = nc.vector.tensor_copy(go[:], gs[k][:])
                else:
                    cp = nc.scalar.copy(go[:], gs[k][:])
                # Tile's auto DMASW wait is satisfied by the prep-time
                # pre-bump, not the gather's completion; the baked per-call
                # sem is the real data-ready signal. The no-sync edge on the
                # trigger keeps the scheduler from hoisting this wait ahead
                # of the phase-1 work on its engine.
                cp._wait_ge(dma_sems[k], 16)
                bass._add_dep_helper(cp.ins, trig_of[k].ins, sync=False,
                                     reason="upconvert after trigger")
                nc.sync.dma_start(
                    out_d[k * IPC:(k + 1) * IPC, :].rearrange(
                        "(g p) d -> p g d", p=P),
                    go[:])
            pbp_cm.__exit__(None, None, None)

    nc.compile()
    return nc


def _shard_inputs(hidden_states, boundary_mask, boundary_prob, take_idx):
    import ml_dtypes
    hidden_states = np.asarray(hidden_states, dtype=np.float32)
    boundary_mask = np.asarray(boundary_mask)
    boundary_prob = np.asarray(boundary_prob, dtype=np.float32)
    take_idx = np.asarray(take_idx)
    ident = np.eye(P, dtype=np.float32)
    identbf = np.eye(P).astype(ml_dtypes.bfloat16)
    uti = np.triu(np.ones((QW, QW), dtype=np.float32))
    in_maps = []
    for b in range(B):
        in_maps.append({
            "ident": ident, "identbf": identbf, "uti": uti,
            "hidden_t": np.ascontiguousarray(hidden_states[b].T),
            # maskw[q, j] = mask[16j + q]
            "maskw": np.ascontiguousarray(
                boundary_mask[b].astype(np.uint8).reshape(FW, QW).T),
            "prob": np.ascontiguousarray(boundary_prob[b].reshape(L, 1)),
            # (p, j) = take_idx[j*128 + p]
            "tidx": np.ascontiguousarray(
                take_idx[b].astype(np.int32).reshape(NPJ, P).T),
        })
    return in_maps


last_results = None  # populated by kernel() for profiling harnesses


def kernel(hidden_states, boundary_mask, boundary_prob, take_idx,
           **run_kwargs) -> np.ndarray:
    global _CACHED_NC, last_results
    if _CACHED_NC is None:
        _CACHED_NC = build_nc()
    in_maps = _shard_inputs(hidden_states, boundary_mask, boundary_prob, take_idx)
    res = run_bass_kernel_spmd(_CACHED_NC, in_maps, core_ids=list(range(B)),
                               **run_kwargs)
    last_results = res
    out = np.stack([np.asarray(res.results[b]["out"]) for b in range(B)], axis=0)
    return out.astype(np.float32, copy=False)
